# revision 3
# baseline (speedup 1.0000x reference)
"""Bass/Trainium2 kernel for the decomposed LocallyConnected2d layer — v5.

out[b,o,i,j] = sum_{c,k} x[b, c, i+di, j+dj] * w[o, c, i, j, k] + bias[o,i,j]
(3x3 kernel, stride 1). Shard over output rows i across 8 cores (4 each).

Same proven fp8 e3m4 matmul core as the 37us baseline (384 x
[96,64]x[96,128], par-packed psum via tile_position, 41t/matmul), with
the data movement rebuilt around the measured ring rates (sync+scalar
HWDGE rings sustain ~265 B/tick aggregate vs 235 for one ring; gpsimd
SWDGE adds ~94 B/tick for 2KB-run descriptors "for free"):

- w rows 0-2 ride the sync ring in six 295KB half-row chunks (host w
  layout [i, jb, 96, dj, 16, o] keeps each chunk contiguous per
  partition); row 3's two chunks ride the scalar ring behind x.
- x stays minimal (0.84MB: xt0 in 3 per-di pieces + rows 3-5, with the
  row-3 piece split out so the xt1 DVE copies can start early); xt1..3
  are built by 9 DVE u32 copies.
- psum->sbuf scaled copies alternate scalar ACT / vector tensor-scalar
  so neither engine queue exceeds the PE phase.
- outputs: rows 0-2 leave via gpsimd SWDGE (full-row 2KB runs),
  row 3 via the scalar ring (idle by then) so the final drain is short.
"""

import sys

for _p in ("/opt/trn_rl_repo", "/root/.axon_site/_ro/trn_rl_repo"):
    if _p not in sys.path:
        sys.path.append(_p)

import numpy as np

B = 128
C_IN = 32
C_OUT = 64
OH = OW = 32
KH = KW = 3
H = W = 34
N_CORES = 8
RPC = OH // N_CORES          # output rows per core = 4
HALO = RPC + KH - 1          # x rows per core = 6
NPAIR = OW // 2              # j-pairs per row = 16
NGRP = 4                     # j-pairs per psum group
GRPS = NPAIR // NGRP         # psum groups per row = 4

WSCALE = 32.0
XSCALE = 2.0

_prog_cache = {}


def _build_program():
    import concourse.tile as tile
    from concourse import bacc, mybir

    f8 = mybir.dt.float8e3
    f32 = mybir.dt.float32
    u32 = mybir.dt.uint32

    nc = bacc.Bacc("TRN2", target_bir_lowering=False, debug=False,
                   num_devices=N_CORES)

    # Per-core DRAM I/O:
    #   x_in [h=6, c=32, w=34, b=128] e3m4 (*2)  (h-major so multi-row
    #     slices map straight onto partitions p = di*32 + c)
    #   w_in [i=4, p=96, jb=2, dj=3, jw=16, o=64] e3m4 (*32); j = 16*jb+jw
    #   out  [p2=128 (par*64+o), i=4, jh=16, b=128] e3m4 ; j = 2*jh + par
    x_in = nc.dram_tensor("x", [HALO, C_IN, W, B], f8,
                          kind="ExternalInput").ap()
    w_in = nc.dram_tensor("w", [RPC, 96, 2, KW, 16, C_OUT], f8,
                          kind="ExternalInput").ap()
    out = nc.dram_tensor("out", [128, RPC, NPAIR, B], f8,
                         kind="ExternalOutput").ap()

    with tile.TileContext(nc) as tc:
        with (
            tc.tile_pool(name="xpool", bufs=1) as xpool,
            tc.tile_pool(name="wpool", bufs=1) as wpool,
            tc.tile_pool(name="opool", bufs=4) as opool,
            tc.tile_pool(name="pspool", bufs=8, space="PSUM") as pspool,
        ):
            xt = [xpool.tile([96, W, B], f8, tag=f"xt{i}",
                             name=f"xt{i}") for i in range(RPC)]
            wt = [wpool.tile([96, 2, KW, 16, C_OUT], f8, tag=f"wt{i}",
                             name=f"wt{i}") for i in range(RPC)]
            xfr3 = xpool.tile([C_IN, W, B], f8, tag="xfr3", name="xfr3")
            xfr45 = xpool.tile([64, W, B], f8, tag="xfr45", name="xfr45")

            # ALL inputs on the single sync HWDGE ring (~235 B/tick when
            # it runs alone) in consumption order — exactly 8 DMAs so
            # each keeps its own completion-semaphore lane. Only row 0's
            # w is split for an earlier first matmul.
            nc.sync.dma_start(xt[0][:], x_in[0:KH])
            nc.sync.dma_start(wt[0][:, 0:1], w_in[0][:, 0:1])
            nc.sync.dma_start(wt[0][:, 1:2], w_in[0][:, 1:2])
            nc.sync.dma_start(xfr3[:], x_in[KH:KH + 1])
            nc.sync.dma_start(wt[1][:], w_in[1])
            nc.sync.dma_start(xfr45[:], x_in[KH + 1:HALO])
            nc.sync.dma_start(wt[2][:], w_in[2])
            nc.sync.dma_start(wt[3][:], w_in[3])

            # Shift copies on DVE (u32 bitcast, 32-partition aligned).
            def xrow(r):
                if r <= 2:
                    return xt[0][32 * r:32 * r + 32, :, :]
                if r == 3:
                    return xfr3[:]
                return xfr45[32 * (r - 4):32 * (r - 4) + 32, :, :]

            for i in range(1, RPC):
                for di in range(KH):
                    nc.vector.tensor_copy(
                        xt[i][32 * di:32 * di + 32, :, :].bitcast(u32),
                        xrow(i + di).bitcast(u32))

            evac = 0
            out_rows = []
            for i in range(RPC):
                out_row = opool.tile([128, NPAIR, B], f8, tag="op")
                out_rows.append(out_row)
                for g in range(GRPS):
                    ps = pspool.tile([128, NGRP, B], f32)
                    for pig in range(NGRP):
                        for par in range(2):
                            j = 2 * (NGRP * g + pig) + par
                            jb, jw = divmod(j, 16)
                            pslice = ps[64 * par:64 * par + 64, pig, :]
                            tp = (0, 64 * par)
                            for dj in range(KW):
                                nc.tensor.matmul(
                                    pslice, wt[i][:, jb, dj, jw, :],
                                    xt[i][:, j + dj, :],
                                    start=(dj == 0), stop=(dj == KW - 1),
                                    tile_position=tp)
                    dst = out_row[:, NGRP * g:NGRP * (g + 1), :]
                    # rows 0-2 evacuate on scalar ACT; row 3 on the DVE
                    # (free after the shift copies) so its evacs overlap
                    # row 2's ACTs and the final drain starts sooner.
                    if i < KH:
                        nc.scalar.mul(dst, ps[:], 1.0 / 64.0)
                    else:
                        nc.vector.tensor_scalar_mul(dst, ps[:], 1.0 / 64.0)
                    evac += 1
            # All output dma_starts are emitted AFTER the matmul loop
            # (their completion-semaphore lanes alias input lanes, and
            # that only raises thresholds for later-emitted instructions
            # — here, just the teardown) and ride the SAME sync queue as
            # the inputs: per-engine in-order descriptor processing means
            # they drain strictly after the last w chunk, so output
            # traffic can never round-robin-steal SDMA bandwidth from
            # the input stream mid-matmul-phase (the root cause of the
            # 46t-vs-37t matmul pacing and the pre-row-3 stall).
            for i in range(RPC):
                nc.sync.dma_start(out[:, i], out_rows[i][:])

    nc.compile()
    return nc


def _host_prep(x, weight):
    """Full fp32 inputs -> list of per-core input dicts."""
    import ml_dtypes

    e3 = ml_dtypes.float8_e3m4
    x_t = np.clip(x.transpose(2, 1, 3, 0) * XSCALE, -15.0, 15.0)
    x_8 = x_t.astype(e3)  # [H, C, W, B]
    w_r = weight.reshape(C_OUT, C_IN, OH, OW, KH, KW)
    w_t = w_r.transpose(2, 4, 1, 5, 3, 0).reshape(OH, 96, KW, OW, C_OUT)
    w_8 = np.clip(w_t * WSCALE, -15.0, 15.0).astype(e3)
    # [I, 96, dj, (jb, jw), O] -> [I, 96, jb, dj, jw, O]
    w_8 = w_8.reshape(OH, 96, KW, 2, 16, C_OUT).transpose(0, 1, 3, 2, 4, 5)

    in_maps = []
    for m in range(N_CORES):
        r0 = m * RPC
        in_maps.append({
            "x": np.ascontiguousarray(x_8[r0:r0 + HALO]),
            "w": np.ascontiguousarray(w_8[r0:r0 + RPC]),
        })
    return in_maps


def _gather(results, bias):
    out_full = np.empty((B, C_OUT, OH, OW), np.float32)
    for m in range(N_CORES):
        r = results[m]["out"].astype(np.float32)          # (128, 4, 16, 128)
        r = r.reshape(2, C_OUT, RPC, NPAIR, B)            # par,o,i,jh,b
        r = r.transpose(4, 1, 2, 3, 0)                    # b,o,i,jh,par
        out_full[:, :, m * RPC:(m + 1) * RPC, :] = r.reshape(B, C_OUT, RPC, OW)
    out_full += bias[None]
    return out_full


def kernel(x, weight, bias, _trace=False):
    from concourse.bass_utils import run_bass_kernel_spmd

    if "nc" not in _prog_cache:
        _prog_cache["nc"] = _build_program()
    nc = _prog_cache["nc"]

    in_maps = _host_prep(np.asarray(x), np.asarray(weight))
    res = run_bass_kernel_spmd(nc, in_maps, core_ids=list(range(N_CORES)),
                               trace=_trace)
    out = _gather(res.results, np.asarray(bias, np.float32))
    if _trace:
        _prog_cache["last_result"] = res
    return out
